# revision 32
# baseline (speedup 1.0000x reference)
"""Trainium2 Bass kernel for CompanySpecificHeads (MoE-style routed MLP heads).

Semantics (matching the reference):
    out[b] = gelu(z[b] @ W1[cid[b]] + b1[cid[b]]) @ W2[cid[b]] + b2[cid[b]]

Strategy: expert-parallel across 8 NeuronCores. Companies are sharded
8-per-core; tokens are routed (gathered by company) to their company's core
on the host, padded to a fixed per-company capacity, and each core runs a
grouped GEMM -> gelu -> dot pipeline over its 8 companies:

  Layer 1 (per company c, h on partitions):
      psum[h, t] = sum_d W1[c][d, h] * zT[c][d, t]      (PE, fp16 operands)
      bias b1 is folded in with a K=4 "selector" matmul that broadcasts
      b1[c][128k+m] across the token axis before accumulation.
  Gelu: ACT engine, PSUM -> SBUF (fp16 out).
  Layer 2: psum2[1, t] += W2[c][hj]^T @ gelu_h[hj, t]   (8 K=128 matmuls)

Host does the unshard/scatter back to [B, 1] and adds b2 (exact, fp32).

DMA discipline: the DIRECT2D DMA encoding supports a single sync wait, so
the kernel keeps every DMACopy at <=1 wait: all loads target fresh SBUF
slots (no reuse -> no release waits), there are <=8 DMAs per DGE flavor
(fresh lane -> no FIFO wait), and the 8 per-company outputs are staged into
one persistent SBUF tile and stored with a single DMA at the end.
"""

import numpy as np

B, C, D, H = 4096, 64, 512, 1024
NCORES = 8
CPC = C // NCORES  # companies per core
KC = D // 128      # contraction chunks of 128
HC = H // 128      # h chunks of 128

_COMPILED = {}


def _build(TW, NTT, dtype_name):
    """Build the Bass/Tile program for per-company token capacity NTT*TW."""
    import concourse.bass as bass
    import concourse.bacc as bacc
    import concourse.mybir as mybir
    from concourse.tile import TileContext
    from contextlib import ExitStack

    f32 = mybir.dt.float32
    dt_op = getattr(mybir.dt, dtype_name)

    SELW = KC * TW           # selector columns
    B1W = CPC * 2 * 128      # b1 columns

    nc = bacc.Bacc(None, target_bir_lowering=False)

    # zt is stored partition-major so one DMA moves it with large packets.
    zt_d = nc.dram_tensor("zt", [128, CPC, NTT, KC, TW], dt_op, kind="ExternalInput")
    # w1 stored as [c][p][g][k][h-half]: a whole company loads linearly
    # with 8KB contiguous per partition (full-rate packets).
    w1_d = nc.dram_tensor(
        "w1", [CPC, 128, 2, KC, H // 2], dt_op, kind="ExternalInput"
    )
    cst_d = nc.dram_tensor("cst", [KC, SELW + B1W], dt_op, kind="ExternalInput")
    w2_d = nc.dram_tensor("w2h", [128, CPC * HC], dt_op, kind="ExternalInput")
    out_d = nc.dram_tensor("out", [1, CPC * NTT * TW], f32, kind="ExternalOutput")

    gelu = mybir.ActivationFunctionType.Gelu

    with TileContext(nc) as tc, ExitStack() as ctx:
        const = ctx.enter_context(tc.tile_pool(name="const", bufs=1))
        # Small constants: [sel | b1h] (4 partitions) and w2 (128 partitions).
        # On the ACT HWDGE ring so they land before w1[0] and the PE can
        # run the bias matmuls while weights stream in.
        ct = const.tile([KC, SELW + B1W], dt_op)
        nc.gpsimd.dma_start(out=ct[:], in_=cst_d[:])
        selt = ct[:, 0:SELW]
        b1t = ct[:, SELW:SELW + B1W].rearrange("p (c g m) -> p c g m", c=CPC, g=2)
        w2t = const.tile([128, CPC * HC], dt_op)
        nc.gpsimd.dma_start(out=w2t[:], in_=w2_d[:])

        # Routed tokens on the ACT HWDGE ring (its dispatch overlaps the SP
        # ring's w1 dispatches): first two companies land early so the PE
        # can start as soon as w1[0] arrives.
        zall = const.tile([128, CPC, NTT, KC, TW], dt_op)
        zsplit = 1
        nc.scalar.dma_start(out=zall[:, :zsplit], in_=zt_d[:, :zsplit])
        if zsplit < CPC:
            nc.scalar.dma_start(out=zall[:, zsplit:], in_=zt_d[:, zsplit:])

        # Staged per-company outputs; single store at the end (SWDGE).
        oall = const.tile([1, CPC * NTT * TW], f32)

        # Per-company weights on the SP HWDGE ring, half a company per DMA.
        # The ring drains FIFO at full bandwidth, so w1 chunks complete in
        # issue order and compute pipelines behind the weight stream.
        w1p = ctx.enter_context(tc.tile_pool(name="w1p", bufs=1))
        w1ts = []
        for c in range(CPC):
            w1t = w1p.tile([128, 2, KC, H // 2], dt_op, name=f"w1_{c}")
            # One DMA per company (8KB contiguous per partition keeps the
            # SP ring at full rate), except the first and last companies,
            # which load in halves: c0's first half lets the PE start ~1us
            # earlier, and c7's first half shortens the pipeline tail.
            if c == 0 or c == CPC - 1:
                nc.sync.dma_start(out=w1t[:, 0], in_=w1_d[c, :, 0])
                nc.sync.dma_start(out=w1t[:, 1], in_=w1_d[c, :, 1])
            else:
                nc.sync.dma_start(out=w1t[:], in_=w1_d[c])
            w1ts.append(w1t)

        hp = ctx.enter_context(tc.tile_pool(name="hp", bufs=min(2 * CPC * NTT, 16)))
        pp = ctx.enter_context(tc.tile_pool(name="pp", bufs=5, space="PSUM"))
        opp = ctx.enter_context(tc.tile_pool(name="opp", bufs=2, space="PSUM"))

        # PE warmup: the PE sits idle ~7us while weights stream in, which
        # drops its clock to 1.2GHz (HAM cold state) right when real work
        # starts. Keep it busy with dependency-free matmuls on scratch data
        # so the first real matmul runs at the warm 2.4GHz clock.
        wsc = const.tile([128, 512], dt_op)
        nc.gpsimd.memset(wsc[:], 0.0)
        wps = ctx.enter_context(tc.tile_pool(name="wps", bufs=1, space="PSUM"))
        wp = wps.tile([128, 512], f32)
        for _ in range(22):
            nc.tensor.matmul(wp[:], wsc[:, :128], wsc[:], start=True, stop=True)

        for c in range(CPC):
            w1t = w1ts[c]
            for tt in range(NTT):
                osum = opp.tile([1, TW], f32)
                for g in range(2):
                    ps = pp.tile([128, KC * TW], f32)
                    # bias: ps[128k+m, (j,t)] = b1[c][512g+128j+m] via selector
                    nc.tensor.matmul(
                        ps[:], b1t[:, c, g, :], selt[:], start=True, stop=False
                    )
                    for j in range(KC):
                        jj = KC * g + j
                        for k in range(KC):
                            nc.tensor.matmul(
                                ps[:, j * TW:(j + 1) * TW],
                                w1t[:, g, k, 128 * j:128 * (j + 1)],
                                zall[:, c, tt, k, :],
                                start=False,
                                stop=(k == KC - 1),
                            )
                    ht = hp.tile([128, KC * TW], dt_op)
                    nc.scalar.activation(ht[:], ps[:], gelu)
                    for j in range(KC):
                        jj = KC * g + j
                        nc.tensor.matmul(
                            osum[:],
                            w2t[:, HC * c + jj:HC * c + jj + 1],
                            ht[:, j * TW:(j + 1) * TW],
                            start=(jj == 0),
                            stop=(jj == HC - 1),
                        )
                off = (c * NTT + tt) * TW
                nc.vector.tensor_copy(oall[:, off:off + TW], osum[:])

        osplit = max(1, (CPC - 2)) * NTT * TW
        nc.gpsimd.dma_start(out=out_d[:, :osplit], in_=oall[:, :osplit])
        nc.gpsimd.dma_start(out=out_d[:, osplit:], in_=oall[:, osplit:])

    nc.finalize()
    return nc


def _get_compiled(TW, NTT, dtype_name):
    key = (TW, NTT, dtype_name)
    if key not in _COMPILED:
        _COMPILED[key] = _build(TW, NTT, dtype_name)
    return _COMPILED[key]


def kernel(z, company_id, W1, b1, W2, b2):
    from concourse.bass_utils import run_bass_kernel_spmd

    z = np.asarray(z, dtype=np.float32)
    cid = np.asarray(company_id).astype(np.int64).ravel()
    W1 = np.asarray(W1, dtype=np.float32)
    b1 = np.asarray(b1, dtype=np.float32)
    W2 = np.asarray(W2, dtype=np.float32)
    b2 = np.asarray(b2, dtype=np.float32)
    O = W2.shape[2]

    np_op = np.float16
    dtype_name = "float16"

    idx_by_company = [np.nonzero(cid == gc)[0] for gc in range(C)]
    max_cnt = max((len(ix) for ix in idx_by_company), default=1)
    max_cnt = max(max_cnt, 1)
    if max_cnt <= 128:
        NTT = 1
        TW = ((max_cnt + 15) // 16) * 16
    else:
        NTT = (max_cnt + 127) // 128
        TW = 128
    CAP = NTT * TW

    nc = _get_compiled(TW, NTT, dtype_name)

    SELW = KC * TW
    B1W = CPC * 2 * 128
    sel = np.repeat(np.eye(KC, dtype=np_op), TW, axis=1)  # [KC, KC*TW]

    in_maps = []
    for core in range(NCORES):
        # zt[p, c, tt, k, t] = z[token, 128k+p]  (partition-major)
        zt = np.zeros((128, CPC, NTT, KC, TW), dtype=np_op)
        for ci in range(CPC):
            gc = core * CPC + ci
            ix = idx_by_company[gc]
            if len(ix) == 0:
                continue
            zpad = np.zeros((CAP, D), dtype=np_op)
            zpad[: len(ix)] = z[ix].astype(np_op)
            zt[:, ci] = zpad.reshape(NTT, TW, KC, 128).transpose(3, 0, 2, 1)
        # w1[c, p, g, k, hh] = W1[gc, 128k+p, 512g+hh]
        w1 = (
            W1[core * CPC:(core + 1) * CPC]
            .reshape(CPC, KC, 128, 2, H // 2)
            .transpose(0, 2, 3, 1, 4)
            .astype(np_op)
        )
        # b1h[k, c, g, m] = b1[gc, 512g+128k+m]
        b1h = (
            b1[core * CPC:(core + 1) * CPC]
            .reshape(CPC, 2, KC, 128)
            .transpose(2, 0, 1, 3)
            .astype(np_op)
        )
        # w2h[p, HC*c + j] = W2[gc, 128j+p, 0]
        w2h = (
            W2[core * CPC:(core + 1) * CPC, :, 0]
            .reshape(CPC, HC, 128)
            .transpose(2, 0, 1)
            .reshape(128, CPC * HC)
            .astype(np_op)
        )
        cst = np.zeros((KC, SELW + B1W), dtype=np_op)
        cst[:, 0:SELW] = sel
        cst[:, SELW:SELW + B1W] = b1h.reshape(KC, B1W)
        in_maps.append(
            {
                "zt": np.ascontiguousarray(zt),
                "w1": np.ascontiguousarray(w1),
                "cst": np.ascontiguousarray(cst),
                "w2h": np.ascontiguousarray(w2h),
            }
        )

    res = run_bass_kernel_spmd(nc, in_maps, list(range(NCORES)))

    out = np.zeros((B, O), dtype=np.float32)
    for core in range(NCORES):
        core_out = res.results[core]["out"].reshape(CPC, NTT * TW)
        for ci in range(CPC):
            gc = core * CPC + ci
            ix = idx_by_company[gc]
            if len(ix) == 0:
                continue
            out[ix, 0] = core_out[ci, : len(ix)] + b2[gc, 0]
    return out


# revision 33
# speedup vs baseline: 1.1949x; 1.1949x over previous
"""Trainium2 Bass kernel for CompanySpecificHeads (MoE-style routed MLP heads).

Semantics (matching the reference):
    out[b] = gelu(z[b] @ W1[cid[b]] + b1[cid[b]]) @ W2[cid[b]] + b2[cid[b]]

Strategy: expert-parallel across 8 NeuronCores. Companies are sharded
8-per-core; tokens are routed (gathered by company) to their company's core
on the host, padded to a fixed per-company capacity, and each core runs a
grouped GEMM -> gelu -> dot pipeline over its 8 companies:

  Layer 1 (per company c, h on partitions):
      psum[h, t] = sum_d W1[c][d, h] * zT[c][d, t]      (PE, fp16 operands)
      bias b1 is folded in with a K=4 "selector" matmul that broadcasts
      b1[c][128k+m] across the token axis before accumulation.
  Gelu: ACT engine, PSUM -> SBUF (fp16 out).
  Layer 2: psum2[1, t] += W2[c][hj]^T @ gelu_h[hj, t]   (8 K=128 matmuls)

Host does the unshard/scatter back to [B, 1] and adds b2 (exact, fp32).

DMA discipline: the DIRECT2D DMA encoding supports a single sync wait, so
the kernel keeps every DMACopy at <=1 wait: all loads target fresh SBUF
slots (no reuse -> no release waits), there are <=8 DMAs per DGE flavor
(fresh lane -> no FIFO wait), and the 8 per-company outputs are staged into
one persistent SBUF tile and stored with a single DMA at the end.
"""

import numpy as np

B, C, D, H = 4096, 64, 512, 1024
NCORES = 8
CPC = C // NCORES  # companies per core
KC = D // 128      # contraction chunks of 128
HC = H // 128      # h chunks of 128

_COMPILED = {}


def _build(TW, NTT, dtype_name):
    """Build the Bass/Tile program for per-company token capacity NTT*TW."""
    import concourse.bass as bass
    import concourse.bacc as bacc
    import concourse.mybir as mybir
    from concourse.tile import TileContext
    from contextlib import ExitStack

    f32 = mybir.dt.float32
    dt_op = getattr(mybir.dt, dtype_name)

    SELW = KC * TW           # selector columns
    B1W = CPC * 2 * 128      # b1 columns

    nc = bacc.Bacc(None, target_bir_lowering=False)

    # zt is stored partition-major so one DMA moves it with large packets.
    zt_d = nc.dram_tensor("zt", [128, CPC, NTT, KC, TW], dt_op, kind="ExternalInput")
    # w1 stored as [c][p][g][k][h-half]: a whole company loads linearly
    # with 8KB contiguous per partition (full-rate packets).
    w1_d = nc.dram_tensor(
        "w1", [CPC, 128, 2, KC, H // 2], dt_op, kind="ExternalInput"
    )
    cst_d = nc.dram_tensor("cst", [KC, SELW + B1W], dt_op, kind="ExternalInput")
    w2_d = nc.dram_tensor("w2h", [128, CPC * HC], dt_op, kind="ExternalInput")
    out_d = nc.dram_tensor("out", [1, CPC * NTT * TW], f32, kind="ExternalOutput")

    gelu = mybir.ActivationFunctionType.Gelu

    with TileContext(nc) as tc, ExitStack() as ctx:
        const = ctx.enter_context(tc.tile_pool(name="const", bufs=1))
        # Small constants: [sel | b1h] (4 partitions) and w2 (128 partitions).
        # On the ACT HWDGE ring so they land before w1[0] and the PE can
        # run the bias matmuls while weights stream in.
        ct = const.tile([KC, SELW + B1W], dt_op)
        nc.gpsimd.dma_start(out=ct[:], in_=cst_d[:])
        selt = ct[:, 0:SELW]
        b1t = ct[:, SELW:SELW + B1W].rearrange("p (c g m) -> p c g m", c=CPC, g=2)
        w2t = const.tile([128, CPC * HC], dt_op)
        nc.gpsimd.dma_start(out=w2t[:], in_=w2_d[:])

        # Routed tokens on the ACT HWDGE ring (its dispatch overlaps the SP
        # ring's w1 dispatches): first two companies land early so the PE
        # can start as soon as w1[0] arrives.
        zall = const.tile([128, CPC, NTT, KC, TW], dt_op)
        zsplit = 1
        nc.scalar.dma_start(out=zall[:, :zsplit], in_=zt_d[:, :zsplit])
        if zsplit < CPC:
            nc.scalar.dma_start(out=zall[:, zsplit:], in_=zt_d[:, zsplit:])

        # Staged per-company outputs; single store at the end (SWDGE).
        oall = const.tile([1, CPC * NTT * TW], f32)

        # Per-company weights on the SP HWDGE ring, half a company per DMA.
        # The ring drains FIFO at full bandwidth, so w1 chunks complete in
        # issue order and compute pipelines behind the weight stream.
        w1p = ctx.enter_context(tc.tile_pool(name="w1p", bufs=1))
        w1ts = []
        for c in range(CPC):
            w1t = w1p.tile([128, 2, KC, H // 2], dt_op, name=f"w1_{c}")
            # One DMA per company: 8KB contiguous per partition keeps the
            # SP ring at full rate (~343 GB/s measured).
            nc.sync.dma_start(out=w1t[:], in_=w1_d[c])
            w1ts.append(w1t)

        hp = ctx.enter_context(tc.tile_pool(name="hp", bufs=min(2 * CPC * NTT, 16)))
        pp = ctx.enter_context(tc.tile_pool(name="pp", bufs=5, space="PSUM"))
        opp = ctx.enter_context(tc.tile_pool(name="opp", bufs=2, space="PSUM"))

        # PE warmup: the PE sits idle ~7us while weights stream in, which
        # drops its clock to 1.2GHz (HAM cold state) right when real work
        # starts. Keep it busy with dependency-free matmuls on scratch data
        # so the first real matmul runs at the warm 2.4GHz clock.
        wsc = const.tile([128, 512], dt_op)
        nc.gpsimd.memset(wsc[:], 0.0)
        wps = ctx.enter_context(tc.tile_pool(name="wps", bufs=1, space="PSUM"))
        wp = wps.tile([128, 512], f32)
        for _ in range(22):
            nc.tensor.matmul(wp[:], wsc[:, :128], wsc[:], start=True, stop=True)

        for c in range(CPC):
            w1t = w1ts[c]
            for tt in range(NTT):
                osum = opp.tile([1, TW], f32)
                for g in range(2):
                    ps = pp.tile([128, KC * TW], f32)
                    # bias: ps[128k+m, (j,t)] = b1[c][512g+128j+m] via selector
                    nc.tensor.matmul(
                        ps[:], b1t[:, c, g, :], selt[:], start=True, stop=False
                    )
                    for j in range(KC):
                        jj = KC * g + j
                        for k in range(KC):
                            nc.tensor.matmul(
                                ps[:, j * TW:(j + 1) * TW],
                                w1t[:, g, k, 128 * j:128 * (j + 1)],
                                zall[:, c, tt, k, :],
                                start=False,
                                stop=(k == KC - 1),
                            )
                    ht = hp.tile([128, KC * TW], dt_op)
                    nc.scalar.activation(ht[:], ps[:], gelu)
                    for j in range(KC):
                        jj = KC * g + j
                        nc.tensor.matmul(
                            osum[:],
                            w2t[:, HC * c + jj:HC * c + jj + 1],
                            ht[:, j * TW:(j + 1) * TW],
                            start=(jj == 0),
                            stop=(jj == HC - 1),
                        )
                off = (c * NTT + tt) * TW
                nc.vector.tensor_copy(oall[:, off:off + TW], osum[:])

        osplit = max(1, (CPC - 2)) * NTT * TW
        nc.gpsimd.dma_start(out=out_d[:, :osplit], in_=oall[:, :osplit])
        nc.gpsimd.dma_start(out=out_d[:, osplit:], in_=oall[:, osplit:])

    nc.finalize()
    return nc


def _get_compiled(TW, NTT, dtype_name):
    key = (TW, NTT, dtype_name)
    if key not in _COMPILED:
        _COMPILED[key] = _build(TW, NTT, dtype_name)
    return _COMPILED[key]


def kernel(z, company_id, W1, b1, W2, b2):
    from concourse.bass_utils import run_bass_kernel_spmd

    z = np.asarray(z, dtype=np.float32)
    cid = np.asarray(company_id).astype(np.int64).ravel()
    W1 = np.asarray(W1, dtype=np.float32)
    b1 = np.asarray(b1, dtype=np.float32)
    W2 = np.asarray(W2, dtype=np.float32)
    b2 = np.asarray(b2, dtype=np.float32)
    O = W2.shape[2]

    np_op = np.float16
    dtype_name = "float16"

    idx_by_company = [np.nonzero(cid == gc)[0] for gc in range(C)]
    max_cnt = max((len(ix) for ix in idx_by_company), default=1)
    max_cnt = max(max_cnt, 1)
    if max_cnt <= 128:
        NTT = 1
        TW = ((max_cnt + 15) // 16) * 16
    else:
        NTT = (max_cnt + 127) // 128
        TW = 128
    CAP = NTT * TW

    nc = _get_compiled(TW, NTT, dtype_name)

    SELW = KC * TW
    B1W = CPC * 2 * 128
    sel = np.repeat(np.eye(KC, dtype=np_op), TW, axis=1)  # [KC, KC*TW]

    in_maps = []
    for core in range(NCORES):
        # zt[p, c, tt, k, t] = z[token, 128k+p]  (partition-major)
        zt = np.zeros((128, CPC, NTT, KC, TW), dtype=np_op)
        for ci in range(CPC):
            gc = core * CPC + ci
            ix = idx_by_company[gc]
            if len(ix) == 0:
                continue
            zpad = np.zeros((CAP, D), dtype=np_op)
            zpad[: len(ix)] = z[ix].astype(np_op)
            zt[:, ci] = zpad.reshape(NTT, TW, KC, 128).transpose(3, 0, 2, 1)
        # w1[c, p, g, k, hh] = W1[gc, 128k+p, 512g+hh]
        w1 = (
            W1[core * CPC:(core + 1) * CPC]
            .reshape(CPC, KC, 128, 2, H // 2)
            .transpose(0, 2, 3, 1, 4)
            .astype(np_op)
        )
        # b1h[k, c, g, m] = b1[gc, 512g+128k+m]
        b1h = (
            b1[core * CPC:(core + 1) * CPC]
            .reshape(CPC, 2, KC, 128)
            .transpose(2, 0, 1, 3)
            .astype(np_op)
        )
        # w2h[p, HC*c + j] = W2[gc, 128j+p, 0]
        w2h = (
            W2[core * CPC:(core + 1) * CPC, :, 0]
            .reshape(CPC, HC, 128)
            .transpose(2, 0, 1)
            .reshape(128, CPC * HC)
            .astype(np_op)
        )
        cst = np.zeros((KC, SELW + B1W), dtype=np_op)
        cst[:, 0:SELW] = sel
        cst[:, SELW:SELW + B1W] = b1h.reshape(KC, B1W)
        in_maps.append(
            {
                "zt": np.ascontiguousarray(zt),
                "w1": np.ascontiguousarray(w1),
                "cst": np.ascontiguousarray(cst),
                "w2h": np.ascontiguousarray(w2h),
            }
        )

    res = run_bass_kernel_spmd(nc, in_maps, list(range(NCORES)))

    out = np.zeros((B, O), dtype=np.float32)
    for core in range(NCORES):
        core_out = res.results[core]["out"].reshape(CPC, NTT * TW)
        for ci in range(CPC):
            gc = core * CPC + ci
            ix = idx_by_company[gc]
            if len(ix) == 0:
                continue
            out[ix, 0] = core_out[ci, : len(ix)] + b2[gc, 0]
    return out


# revision 35
# speedup vs baseline: 1.2415x; 1.0390x over previous
"""Trainium2 Bass kernel for CompanySpecificHeads (MoE-style routed MLP heads).

Semantics (matching the reference):
    out[b] = gelu(z[b] @ W1[cid[b]] + b1[cid[b]]) @ W2[cid[b]] + b2[cid[b]]

Strategy: expert-parallel across 8 NeuronCores. Companies are sharded
8-per-core; tokens are routed (gathered by company) to their company's core
on the host, padded to a fixed per-company capacity, and each core runs a
grouped GEMM -> gelu -> dot pipeline over its 8 companies:

  Layer 1 (per company c, h on partitions):
      psum[h, t] = sum_d W1[c][d, h] * zT[c][d, t]      (PE, fp16 operands)
      bias b1 is folded in with a K=4 "selector" matmul that broadcasts
      b1[c][128k+m] across the token axis before accumulation.
  Gelu: ACT engine, PSUM -> SBUF (fp16 out).
  Layer 2: psum2[1, t] += W2[c][hj]^T @ gelu_h[hj, t]   (8 K=128 matmuls)

Host does the unshard/scatter back to [B, 1] and adds b2 (exact, fp32).

DMA discipline: the DIRECT2D DMA encoding supports a single sync wait, so
the kernel keeps every DMACopy at <=1 wait: all loads target fresh SBUF
slots (no reuse -> no release waits), there are <=8 DMAs per DGE flavor
(fresh lane -> no FIFO wait), and the 8 per-company outputs are staged into
one persistent SBUF tile and stored with a single DMA at the end.
"""

import numpy as np

B, C, D, H = 4096, 64, 512, 1024
NCORES = 8
CPC = C // NCORES  # companies per core
KC = D // 128      # contraction chunks of 128
HC = H // 128      # h chunks of 128

_COMPILED = {}


def _build(TW, NTT, dtype_name):
    """Build the Bass/Tile program for per-company token capacity NTT*TW."""
    import concourse.bass as bass
    import concourse.bacc as bacc
    import concourse.mybir as mybir
    from concourse.tile import TileContext
    from contextlib import ExitStack

    f32 = mybir.dt.float32
    dt_op = getattr(mybir.dt, dtype_name)

    SELW = KC * TW           # selector columns
    B1W = CPC * 2 * 128      # b1 columns

    nc = bacc.Bacc(None, target_bir_lowering=False)

    # zt is stored partition-major so one DMA moves it with large packets.
    zt_d = nc.dram_tensor("zt", [128, CPC, NTT, KC, TW], dt_op, kind="ExternalInput")
    # w1 stored as [c][p][g][k][h-half]: a whole company loads linearly
    # with 8KB contiguous per partition (full-rate packets).
    w1_d = nc.dram_tensor(
        "w1", [CPC, 128, 2, KC, H // 2], dt_op, kind="ExternalInput"
    )
    cst_d = nc.dram_tensor("cst", [KC, SELW + B1W], dt_op, kind="ExternalInput")
    w2_d = nc.dram_tensor("w2h", [128, CPC * HC], dt_op, kind="ExternalInput")
    out_d = nc.dram_tensor("out", [1, CPC * NTT * TW], f32, kind="ExternalOutput")

    gelu = mybir.ActivationFunctionType.Gelu

    with TileContext(nc) as tc, ExitStack() as ctx:
        const = ctx.enter_context(tc.tile_pool(name="const", bufs=1))
        # Small constants: [sel | b1h] (4 partitions) and w2 (128 partitions).
        # On the ACT HWDGE ring so they land before w1[0] and the PE can
        # run the bias matmuls while weights stream in.
        ct = const.tile([KC, SELW + B1W], dt_op)
        nc.gpsimd.dma_start(out=ct[:], in_=cst_d[:])
        selt = ct[:, 0:SELW]
        b1t = ct[:, SELW:SELW + B1W].rearrange("p (c g m) -> p c g m", c=CPC, g=2)
        w2t = const.tile([128, CPC * HC], dt_op)
        nc.gpsimd.dma_start(out=w2t[:], in_=w2_d[:])

        # Routed tokens on the ACT HWDGE ring (its dispatch overlaps the SP
        # ring's w1 dispatches): first two companies land early so the PE
        # can start as soon as w1[0] arrives.
        zall = const.tile([128, CPC, NTT, KC, TW], dt_op)
        zsplit = 1
        nc.scalar.dma_start(out=zall[:, :zsplit], in_=zt_d[:, :zsplit])
        if zsplit < CPC:
            nc.scalar.dma_start(out=zall[:, zsplit:], in_=zt_d[:, zsplit:])

        # Staged per-company outputs; single store at the end (SWDGE).
        oall = const.tile([1, CPC * NTT * TW], f32)

        # Per-company weights on the SP HWDGE ring, half a company per DMA.
        # The ring drains FIFO at full bandwidth, so w1 chunks complete in
        # issue order and compute pipelines behind the weight stream.
        w1p = ctx.enter_context(tc.tile_pool(name="w1p", bufs=1))
        w1ts = []
        for c in range(CPC):
            w1t = w1p.tile([128, 2, KC, H // 2], dt_op, name=f"w1_{c}")
            # One DMA per company: 8KB contiguous per partition keeps the
            # SP ring at full rate (~343 GB/s measured).
            nc.sync.dma_start(out=w1t[:], in_=w1_d[c])
            w1ts.append(w1t)

        hp = ctx.enter_context(tc.tile_pool(name="hp", bufs=min(2 * CPC * NTT, 16)))
        pp = ctx.enter_context(tc.tile_pool(name="pp", bufs=5, space="PSUM"))
        opp = ctx.enter_context(tc.tile_pool(name="opp", bufs=2, space="PSUM"))

        # PE warmup: the PE sits idle ~7us while weights stream in, which
        # drops its clock to 1.2GHz (HAM cold state) right when real work
        # starts. Keep it busy with dependency-free matmuls on scratch data
        # so the first real matmul runs at the warm 2.4GHz clock.
        wsc = const.tile([128, 512], dt_op)
        nc.gpsimd.memset(wsc[:], 0.0)
        wps = ctx.enter_context(tc.tile_pool(name="wps", bufs=1, space="PSUM"))
        wp = wps.tile([128, 512], f32)
        for _ in range(22):
            nc.tensor.matmul(wp[:], wsc[:, :128], wsc[:], start=True, stop=True)

        for c in range(CPC):
            w1t = w1ts[c]
            for tt in range(NTT):
                osum = opp.tile([1, TW], f32)
                for g in range(2):
                    ps = pp.tile([128, KC * TW], f32)
                    # bias: ps[128k+m, (j,t)] = b1[c][512g+128j+m] via selector
                    nc.tensor.matmul(
                        ps[:], b1t[:, c, g, :], selt[:], start=True, stop=False
                    )
                    for j in range(KC):
                        jj = KC * g + j
                        for k in range(KC):
                            nc.tensor.matmul(
                                ps[:, j * TW:(j + 1) * TW],
                                w1t[:, g, k, 128 * j:128 * (j + 1)],
                                zall[:, c, tt, k, :],
                                start=False,
                                stop=(k == KC - 1),
                            )
                    ht = hp.tile([128, KC * TW], dt_op)
                    nc.scalar.activation(ht[:], ps[:], gelu)
                    for j in range(KC):
                        jj = KC * g + j
                        nc.tensor.matmul(
                            osum[:],
                            w2t[:, HC * c + jj:HC * c + jj + 1],
                            ht[:, j * TW:(j + 1) * TW],
                            start=(jj == 0),
                            stop=(jj == HC - 1),
                        )
                off = (c * NTT + tt) * TW
                nc.vector.tensor_copy(oall[:, off:off + TW], osum[:])

        osplit = max(1, (CPC - 2)) * NTT * TW
        nc.gpsimd.dma_start(out=out_d[:, :osplit], in_=oall[:, :osplit])
        nc.gpsimd.dma_start(out=out_d[:, osplit:], in_=oall[:, osplit:])

    nc.finalize()
    return nc


def _get_compiled(TW, NTT, dtype_name):
    key = (TW, NTT, dtype_name)
    if key not in _COMPILED:
        _COMPILED[key] = _build(TW, NTT, dtype_name)
    return _COMPILED[key]


def kernel(z, company_id, W1, b1, W2, b2):
    from concourse.bass_utils import run_bass_kernel_spmd

    z = np.asarray(z, dtype=np.float32)
    cid = np.asarray(company_id).astype(np.int64).ravel()
    W1 = np.asarray(W1, dtype=np.float32)
    b1 = np.asarray(b1, dtype=np.float32)
    W2 = np.asarray(W2, dtype=np.float32)
    b2 = np.asarray(b2, dtype=np.float32)
    O = W2.shape[2]

    np_op = np.float16
    dtype_name = "float16"

    idx_by_company = [np.nonzero(cid == gc)[0] for gc in range(C)]
    max_cnt = max((len(ix) for ix in idx_by_company), default=1)
    max_cnt = max(max_cnt, 1)
    if max_cnt <= 128:
        NTT = 1
        TW = ((max_cnt + 15) // 16) * 16
    else:
        NTT = (max_cnt + 127) // 128
        TW = 128
    CAP = NTT * TW

    nc = _get_compiled(TW, NTT, dtype_name)

    SELW = KC * TW
    B1W = CPC * 2 * 128
    sel = np.repeat(np.eye(KC, dtype=np_op), TW, axis=1)  # [KC, KC*TW]

    in_maps = []
    for core in range(NCORES):
        # zt[p, c, tt, k, t] = z[token, 128k+p]  (partition-major)
        zt = np.zeros((128, CPC, NTT, KC, TW), dtype=np_op)
        for ci in range(CPC):
            gc = core * CPC + ci
            ix = idx_by_company[gc]
            if len(ix) == 0:
                continue
            zpad = np.zeros((CAP, D), dtype=np_op)
            zpad[: len(ix)] = z[ix].astype(np_op)
            zt[:, ci] = zpad.reshape(NTT, TW, KC, 128).transpose(3, 0, 2, 1)
        # w1[c, p, g, k, hh] = W1[gc, 128k+p, 512g+hh]
        w1 = (
            W1[core * CPC:(core + 1) * CPC]
            .reshape(CPC, KC, 128, 2, H // 2)
            .transpose(0, 2, 3, 1, 4)
            .astype(np_op)
        )
        # b1h[k, c, g, m] = b1[gc, 512g+128k+m]
        b1h = (
            b1[core * CPC:(core + 1) * CPC]
            .reshape(CPC, 2, KC, 128)
            .transpose(2, 0, 1, 3)
            .astype(np_op)
        )
        # w2h[p, HC*c + j] = W2[gc, 128j+p, 0]
        w2h = (
            W2[core * CPC:(core + 1) * CPC, :, 0]
            .reshape(CPC, HC, 128)
            .transpose(2, 0, 1)
            .reshape(128, CPC * HC)
            .astype(np_op)
        )
        cst = np.zeros((KC, SELW + B1W), dtype=np_op)
        cst[:, 0:SELW] = sel
        cst[:, SELW:SELW + B1W] = b1h.reshape(KC, B1W)
        in_maps.append(
            {
                "zt": np.ascontiguousarray(zt),
                "w1": np.ascontiguousarray(w1),
                "cst": np.ascontiguousarray(cst),
                "w2h": np.ascontiguousarray(w2h),
            }
        )

    res = run_bass_kernel_spmd(nc, in_maps, list(range(NCORES)))

    out = np.zeros((B, O), dtype=np.float32)
    for core in range(NCORES):
        core_out = res.results[core]["out"].reshape(CPC, NTT * TW)
        for ci in range(CPC):
            gc = core * CPC + ci
            ix = idx_by_company[gc]
            if len(ix) == 0:
                continue
            out[ix, 0] = core_out[ci, : len(ix)] + b2[gc, 0]
    return out
